# revision 31
# baseline (speedup 1.0000x reference)
"""GRU-ODE (Neural ODE, dopri5 reference) Trainium2 kernel.

Contract: kernel(**inputs) takes FULL inputs (x0 [1024,1024], t [16],
W_hr/W_hz/W_hh [1024,1024], all fp32) and returns the FULL output
[1024, 16, 1024] fp32, matching
    odeint(f, x0, t, rtol=1e-5, atol=1e-6)  (dopri5)  transposed to [B,T,H]
with f(h) = (1-sigmoid(h@Wz.T)) * (tanh((sigmoid(h@Wr.T)*h)@Wh.T) - h).

Strategy: data-parallel over batch across 8 NeuronCores (128 rows/core —
exactly the SBUF partition width). Each core integrates its shard
independently (no collectives): fixed-step RK4 with N_BIG=2 uneven big
steps (nodes at fractions 0, 0.6, 1.0 of the span — the short last step
leaves only 5 dense-output points depending on the final derivative,
which otherwise dominate the kernel tail) plus cubic-Hermite dense output
at the 16 requested times. Scheme error vs the adaptive dopri5 reference
is ~7e-5 rel; bf16 matmul rounding dominates at ~1.7e-4 rel / ~1.5e-3
absmax.

Performance structure (per core, per f-eval): 48 bf16 matmuls
[128x128]x[128x512] accumulating over 8 K-chunks into PSUM, plus 16 PE
transposes (128x128) to build the transposed stationary operands. The
serial inter-eval dependency (tanh -> k -> state update -> transpose) is
algebraically shortened: with p = c*sigmoid(-a_z) and q = h - p*y
precomputed off the critical path, the next stage state is just
y_next = q + p*tanh(a_u), i.e. two vector ops after the tanh. The RK4
combination is likewise folded into the final stage:
  h_new = (y2 + 2*y3 + y4 - h)/3 + (dt/6)*s4*(u4 - y4)
        = G + p4*u4   with G precomputed off-path.
All elementwise tail work runs at half-width (512 cols) so the next
eval's matmuls can start as soon as the first half of the transposed
state lands.
"""

import numpy as np

import concourse.bacc as bacc
import concourse.bass as bass
import concourse.mybir as mybir
import concourse.tile as tile
from concourse import bass_utils

B, H, T = 1024, 1024, 16
N_CORES = 8
BS = B // N_CORES  # 128 batch rows per core
N_BIG = 2          # RK4 big steps across [t0, t_last]
P = 128
NK = H // P        # 8 contraction chunks
NO = H // 512      # 2 psum output chunks

F32 = mybir.dt.float32
F16 = mybir.dt.float16
BF16 = mybir.dt.bfloat16
AF = mybir.ActivationFunctionType
ALU = mybir.AluOpType

# set by the dev harness (test.py) only; grading uses the defaults
TRACE = False
TRACE_DIR = None
LAST_EXEC_NS = None


def _build_program(t_vals: np.ndarray):
    """Build the SPMD Bass/Tile program (same on every core)."""
    t0 = float(t_vals[0])
    t_end = float(t_vals[-1])
    # uneven big steps: the last step is short so few dense-output points
    # depend on the final derivative (they dominate the kernel tail)
    FRACS = [0.0, 0.6, 1.0]
    nodes_t = [t0 + f * (t_end - t0) for f in FRACS]
    Hsteps = [nodes_t[s + 1] - nodes_t[s] for s in range(N_BIG)]

    # map each output index j>0 to (step s, tau in (0,1]); tau==1 -> node
    out_plan = {s: [] for s in range(N_BIG)}
    node_out = {}  # step s whose END node is output index j
    for j in range(1, T):
        tj = float(t_vals[j])
        s = max(i for i in range(N_BIG) if nodes_t[i] <= tj + 1e-9)
        s = min(s, N_BIG - 1)
        tau = (tj - nodes_t[s]) / Hsteps[s]
        if tau >= 1.0 - 1e-9:
            node_out[s] = j
        else:
            out_plan[s].append((j, tau))

    nc = bacc.Bacc("TRN2", target_bir_lowering=False, debug=False)

    x0_d = nc.dram_tensor("x0s", [BS, H], F32, kind="ExternalInput")
    wr_d = nc.dram_tensor("WrT", [H, H], BF16, kind="ExternalInput")
    wz_d = nc.dram_tensor("WzT", [H, H], BF16, kind="ExternalInput")
    wh_d = nc.dram_tensor("WhT", [H, H], BF16, kind="ExternalInput")
    id_d = nc.dram_tensor("ident", [P, P], F32, kind="ExternalInput")
    out_d = nc.dram_tensor("out", [T, BS, H], F32, kind="ExternalOutput")

    HALF = H // 2  # 512

    def halves(tile_, no):
        return tile_[:, no * HALF:(no + 1) * HALF]

    with tile.TileContext(nc) as tc:
        with (
            tc.tile_pool(name="wpool", bufs=1) as wpool,
            tc.tile_pool(name="state", bufs=1) as state,
            tc.tile_pool(name="work", bufs=1) as work,
            tc.tile_pool(name="psA", bufs=6, space="PSUM") as psA,
            tc.tile_pool(name="psT", bufs=2, space="PSUM") as psT,
        ):
            # --- inputs: x0 and identity first so PE can start early ----
            h0_sb = state.tile([BS, H], F32, tag="node0")
            nc.sync.dma_start(h0_sb[:, :H // 2], x0_d[:, :H // 2])
            nc.sync.dma_start(h0_sb[:, H // 2:], x0_d[:, H // 2:])
            ident = wpool.tile([P, P], F32, tag="ident")
            nc.sync.dma_start(ident[:], id_d[:, :])
            # weights in 2-chunk pieces so the first matmuls start while
            # the rest still streams
            w_sb = {}
            for nm, dram, eng in (("r", wr_d, nc.sync), ("z", wz_d, nc.sync),
                                  ("h", wh_d, nc.sync)):
                wt = wpool.tile([P, NK, H], BF16, tag=f"w_{nm}")
                dv = dram.rearrange("(kc p) h -> p kc h", p=P)
                for c0 in range(0, NK, 2):
                    eng.dma_start(wt[:, c0:c0 + 2, :], dv[:, c0:c0 + 2, :])
                w_sb[nm] = wt

            # out[0] = x0 exactly (after the weight DMAs in queue order so
            # it does not delay them)
            nc.sync.dma_start(out_d[0, :, :], h0_sb[:])

            # --- helpers ------------------------------------------------
            def transpose_half(dst_sb, src_sb, no):
                """dst_sb[:, no*512 : ...] = blockwise-transposed half of
                src_sb (chunks kc = 4*no .. 4*no+3)."""
                pst = psT.tile([P, HALF], F32, tag="pst", name=f"pst_{no}")
                for c in range(4):
                    kc = no * 4 + c
                    nc.tensor.transpose(
                        pst[:, c * P:(c + 1) * P],
                        src_sb[:, kc * P:(kc + 1) * P],
                        ident[:],
                    )
                nc.scalar.copy(halves(dst_sb, no), pst[:])

            def matmul_group(ps_tile, yT, w, no):
                for kc in range(NK):
                    nc.tensor.matmul(
                        ps_tile[:],
                        yT[:, kc * P:(kc + 1) * P],
                        w[:, kc, no * HALF:(no + 1) * HALF],
                        start=(kc == 0),
                        stop=(kc == NK - 1),
                    )

            def eval_f(y_sb, yT, name, tail_cb, mid_cb=None):
                """One f evaluation at state y_sb (with its transposed bf16
                copy yT already in SBUF). Emission order matters: engines
                execute in-order, so sigmoids come before the PSUM->SBUF
                copies, off-path work (mid_cb) goes before the a_u matmuls,
                and both tanhs precede the tail callbacks."""
                a_r = [psA.tile([P, HALF], F32, tag="psA", name=f"ar{name}{o}")
                       for o in range(NO)]
                for no in range(NO):
                    matmul_group(a_r[no], yT, w_sb["r"], no)
                a_z = [psA.tile([P, HALF], F32, tag="psA", name=f"az{name}{o}")
                       for o in range(NO)]
                for no in range(NO):
                    matmul_group(a_z[no], yT, w_sb["z"], no)

                r = work.tile([BS, H], F32, tag="r")
                sneg = work.tile([BS, H], F32, tag="sneg", bufs=2)
                rh = work.tile([BS, H], F32, tag="rh")
                rhT = work.tile([BS, H], BF16, tag="rhT")
                for no in range(NO):
                    nc.scalar.activation(halves(r, no), a_r[no][:], AF.Sigmoid)
                for no in range(NO):
                    nc.scalar.activation(halves(sneg, no), a_z[no][:],
                                         AF.Sigmoid, scale=-1.0)
                for no in range(NO):
                    nc.vector.tensor_mul(halves(rh, no), halves(r, no),
                                         halves(y_sb, no))
                for no in range(NO):
                    transpose_half(rhT, rh, no)

                if mid_cb is not None:
                    mid_cb(sneg)

                a_u = [psA.tile([P, HALF], F32, tag="psA", name=f"au{name}{o}")
                       for o in range(NO)]
                for no in range(NO):
                    matmul_group(a_u[no], rhT, w_sb["h"], no)
                u = work.tile([BS, H], F32, tag="u", bufs=2)
                for no in range(NO):
                    nc.scalar.activation(halves(u, no), a_u[no][:], AF.Tanh)
                for no in range(NO):
                    tail_cb(no, u, sneg)
                return u, sneg

            # --- dense-output helpers -----------------------------------
            # Hermite p(tau) = y0 + h01*(y1-y0) + h10*f0 + h11*f1
            # Engines execute in emission order, so interpolation work is
            # drained in small chunks right after each eval's critical ops.
            interp_state = {}

            def interp_coeffs(s, tau):
                Hs = Hsteps[s]
                t2, t3 = tau * tau, tau ** 3
                return (-2 * t3 + 3 * t2, (t3 - 2 * t2 + tau) * Hs,
                        (t3 - t2) * Hs)

            def interp_make_D(s):
                Dt = state.tile([BS, H], F32, tag=f"D{s}", name=f"D{s}")
                nc.vector.tensor_sub(Dt[:], node[s + 1][:], node[s][:])
                interp_state[s] = Dt

            def interp_point(s, j, tau):
                """3 vector ops + DMA for one dense-output point."""
                h01, h10, h11 = interp_coeffs(s, tau)
                Dt = interp_state[s]
                acc = work.tile([BS, H], F32, tag="interp", bufs=2,
                                name=f"acc_{s}_{j}")
                nc.vector.scalar_tensor_tensor(
                    acc[:], Dt[:], h01, node[s][:], ALU.mult, ALU.add)
                nc.vector.scalar_tensor_tensor(
                    acc[:], fnode[s][:], h10, acc[:], ALU.mult, ALU.add)
                nc.vector.scalar_tensor_tensor(
                    acc[:], fnode[s + 1][:], h11, acc[:], ALU.mult, ALU.add)
                nc.sync.dma_start(out_d[j, :, :], acc[:])

            pending = []  # (s, j, tau) interp points ready to drain

            def drain_interp(n):
                for _ in range(min(n, len(pending))):
                    interp_point(*pending.pop(0))

            # --- integration --------------------------------------------
            # all nodes/derivatives stay live for the Hermite dense output
            node = [h0_sb] + [
                state.tile([BS, H], F32, tag=f"node{s + 1}", name=f"node{s + 1}")
                for s in range(N_BIG)]
            fnode = [
                state.tile([BS, H], F32, tag=f"fn{s}", name=f"fn{s}")
                for s in range(N_BIG + 1)]

            # initial transposed state
            hT0 = work.tile([BS, H], BF16, tag="yT", name="hT0", bufs=2)
            for no in range(NO):
                transpose_half(hT0, h0_sb, no)

            def make_stage_tail(p_t, q_t, y_new, yT_new):
                """tail: y_new = q + p*u per half, then transpose+copy."""
                def cb(no, u, sneg):
                    tmp = work.tile([BS, H], F32, tag="ttmp", bufs=2,
                                    name=f"tt{id(u)}{no}")
                    nc.vector.tensor_mul(halves(tmp, no), halves(p_t, no),
                                         halves(u, no))
                    nc.vector.tensor_add(halves(y_new, no), halves(q_t, no),
                                         halves(tmp, no))
                    transpose_half(yT_new, y_new, no)
                return cb

            def emit_pq(p_t, q_t, sneg, y_sb, h_sb, c):
                """off-path: p = c*sneg (ACT); q = h - p*y.
                When y is h itself (stage 1), q = (1-p)*h with the (1-p)
                computed as a second ACT copy: one DVE op instead of two."""
                nc.scalar.activation(p_t[:], sneg[:], AF.Copy, scale=float(c))
                if y_sb is h_sb:
                    m = work.tile([BS, H], F32, tag="gtmp")
                    nc.scalar.activation(m[:], sneg[:], AF.Copy,
                                         scale=float(-c), bias=1.0)
                    nc.vector.tensor_mul(q_t[:], m[:], h_sb[:])
                else:
                    g = work.tile([BS, H], F32, tag="gtmp")
                    nc.vector.tensor_mul(g[:], p_t[:], y_sb[:])
                    nc.vector.scalar_tensor_tensor(
                        q_t[:], g[:], -1.0, h_sb[:], ALU.mult, ALU.add)

            def emit_fnode(f_t, u, sneg, y_sb):
                """off-path: f = (u - y) * sneg (for Hermite)"""
                d = work.tile([BS, H], F32, tag="fd")
                nc.vector.tensor_sub(d[:], u[:], y_sb[:])
                nc.vector.tensor_mul(f_t[:], d[:], sneg[:])

            # E0: f(x0)
            cur_y, cur_yT = h0_sb, hT0

            # interleaved stepping
            for s in range(N_BIG):
                dt = Hsteps[s]
                h_sb = node[s]
                h_new = node[s + 1]

                y2 = work.tile([BS, H], F32, tag="y2", name=f"y2_{s}")
                y2T = work.tile([BS, H], BF16, tag="yT", name=f"y2T_{s}",
                                bufs=2)
                p1 = work.tile([BS, H], F32, tag="p", name=f"p1_{s}", bufs=2)
                q1 = work.tile([BS, H], F32, tag="q", name=f"q1_{s}", bufs=2)

                # E1 at the node: tail builds y2 = h + (dt/2)*k1
                u1, s1 = eval_f(
                    cur_y, cur_yT, f"e1s{s}",
                    make_stage_tail(p1, q1, y2, y2T),
                    mid_cb=lambda sneg, _h=h_sb: emit_pq(
                        p1, q1, sneg, _h, _h, dt / 2))
                emit_fnode(fnode[s], u1, s1, h_sb)
                drain_interp(3)

                # E2 at y2 -> y3 = h + (dt/2)*k2
                y3 = work.tile([BS, H], F32, tag="y3", name=f"y3_{s}")
                y3T = work.tile([BS, H], BF16, tag="yT", name=f"y3T_{s}",
                                bufs=2)
                p2 = work.tile([BS, H], F32, tag="p", name=f"p2_{s}", bufs=2)
                q2 = work.tile([BS, H], F32, tag="q", name=f"q2_{s}", bufs=2)
                eval_f(
                    y2, y2T, f"e2s{s}",
                    make_stage_tail(p2, q2, y3, y3T),
                    mid_cb=lambda sneg, _h=h_sb, _y=y2: emit_pq(
                        p2, q2, sneg, _y, _h, dt / 2))
                drain_interp(3)

                # E3 at y3 -> y4 = h + dt*k3; also start the RK4-combination
                # chain m1 = y2 - h (ready input, runs in E3's slack)
                y4 = work.tile([BS, H], F32, tag="y4", name=f"y4_{s}")
                y4T = work.tile([BS, H], BF16, tag="yT", name=f"y4T_{s}",
                                bufs=2)
                p3 = work.tile([BS, H], F32, tag="p", name=f"p3_{s}", bufs=2)
                q3 = work.tile([BS, H], F32, tag="q", name=f"q3_{s}", bufs=2)
                m1 = work.tile([BS, H], F32, tag="m1", name=f"m1_{s}")

                def mid3(sneg, _h=h_sb, _y=y3, _m1=m1, _y2=y2, _p=p3, _q=q3,
                         _dt=dt):
                    emit_pq(_p, _q, sneg, _y, _h, _dt)
                    nc.vector.scalar_tensor_tensor(
                        _m1[:], _h[:], -1.0, _y2[:], ALU.mult, ALU.add)

                eval_f(y3, y3T, f"e3s{s}", make_stage_tail(p3, q3, y4, y4T),
                       mid_cb=mid3)
                drain_interp(2)

                # E4 at y4 -> h_new = (y2+2y3+y4-h)/3 + (dt/6)*k4 = G + p4*u4
                hnT = work.tile([BS, H], BF16, tag="yT", name=f"hnT_{s}",
                                bufs=2)
                p4 = work.tile([BS, H], F32, tag="p", name=f"p4_{s}", bufs=2)
                G = work.tile([BS, H], F32, tag="q", name=f"G_{s}", bufs=2)

                def mid4(sneg, _h=h_sb, _p=p4, _G=G, _m1=m1, _y3=y3, _y4=y4,
                         _dt=dt, _s=s):
                    nc.scalar.activation(_p[:], sneg[:], AF.Copy,
                                         scale=float(_dt / 6))
                    m2 = work.tile([BS, H], F32, tag="gtmp", name=f"m2_{_s}")
                    nc.vector.scalar_tensor_tensor(
                        m2[:], _y3[:], 2.0, _m1[:], ALU.mult, ALU.add)
                    nc.vector.scalar_tensor_tensor(
                        m2[:], _y4[:], 1.0, m2[:], ALU.mult, ALU.add)
                    g4 = work.tile([BS, H], F32, tag="gtmp2", name=f"g4_{_s}")
                    nc.vector.tensor_mul(g4[:], _p[:], _y4[:])
                    nc.vector.scalar_tensor_tensor(
                        _G[:], m2[:], 1.0 / 3.0, g4[:], ALU.mult,
                        ALU.subtract)

                eval_f(y4, y4T, f"e4s{s}", make_stage_tail(p4, G, h_new, hnT),
                       mid_cb=mid4)
                drain_interp(2)

                cur_y, cur_yT = h_new, hnT
                interp_make_D(s)
                if s < N_BIG - 1:
                    pending.extend((s, j, tau) for (j, tau) in out_plan[s])

                # node output DMA
                if s in node_out:
                    nc.sync.dma_start(out_d[node_out[s], :, :], h_new[:])

            def tail_noop(no, u, sneg):
                pass

            uF, sF = eval_f(cur_y, cur_yT, "efin", tail_noop)

            # drain whatever interpolation is still pending for earlier
            # steps, and precompute the last step's partial sums
            # pre_j = y0 + h01*D + h10*f0 (they only need node data), so
            # after the final derivative lands each output is ONE more op.
            sL = N_BIG - 1
            pres = []
            for (j, tau) in out_plan[sL]:
                h01, h10, h11 = interp_coeffs(sL, tau)
                pre = work.tile([BS, H], F32, tag=f"pre{j}", name=f"pre{j}")
                nc.vector.scalar_tensor_tensor(
                    pre[:], interp_state[sL][:], h01, node[sL][:],
                    ALU.mult, ALU.add)
                nc.vector.scalar_tensor_tensor(
                    pre[:], fnode[sL][:], h10, pre[:], ALU.mult, ALU.add)
                pres.append((j, h11, pre))
            drain_interp(99)

            emit_fnode(fnode[N_BIG], uF, sF, cur_y)
            for (j, h11, pre) in pres:
                accf = work.tile([BS, H], F32, tag="interp", bufs=2,
                                 name=f"accf{j}")
                nc.vector.scalar_tensor_tensor(
                    accf[:], fnode[N_BIG][:], h11, pre[:], ALU.mult, ALU.add)
                nc.sync.dma_start(out_d[j, :, :], accf[:])

            # (dense output handled inline above; see emit helpers)

    nc.compile()
    return nc


def kernel(x0, t, W_hr, W_hz, W_hh):
    x0 = np.ascontiguousarray(np.asarray(x0, dtype=np.float32))
    t = np.asarray(t, dtype=np.float32)
    import ml_dtypes
    bf = ml_dtypes.bfloat16
    WrT = np.ascontiguousarray(np.asarray(W_hr, dtype=np.float32).T.astype(bf))
    WzT = np.ascontiguousarray(np.asarray(W_hz, dtype=np.float32).T.astype(bf))
    WhT = np.ascontiguousarray(np.asarray(W_hh, dtype=np.float32).T.astype(bf))
    ident = np.eye(P, dtype=np.float32)

    nc = _build_program(t)

    in_maps = []
    for c in range(N_CORES):
        in_maps.append({
            "x0s": x0[c * BS:(c + 1) * BS],
            "WrT": WrT, "WzT": WzT, "WhT": WhT,
            "ident": ident,
        })
    kw = {}
    if TRACE:
        kw = dict(trace=True, tmpdir=TRACE_DIR)
    res = bass_utils.run_bass_kernel_spmd(
        nc, in_maps, core_ids=list(range(N_CORES)), **kw)
    global LAST_EXEC_NS
    LAST_EXEC_NS = res.exec_time_ns
    # res.results[c]["out"] : [T, BS, H]
    full = np.concatenate([res.results[c]["out"] for c in range(N_CORES)], axis=1)
    return np.ascontiguousarray(full.transpose(1, 0, 2))
